# revision 44
# baseline (speedup 1.0000x reference)
"""Causal multi-head attention on 8 Trainium2 NeuronCores.

Sharding: core c -> batch (c // 4), head-group (c % 4) of 4 heads
(tensor-parallel over the 16 heads, data-parallel over batch=2).
Each core computes its 4 heads' contribution to the output projection;
the host sums the 4 per-head-group partials per batch (the "all-reduce")
and adds b_O.

Kernel layout notes (per core):
  - everything is computed in transposed [feature, seq] layout so the
    softmax reduction (over keys) lands on PSUM partitions and can be
    done with a ones-matmul on the PE.
  - all matmul operands are bf16 (PSUM accumulation fp32); softmax
    numerator (P@V) and denominator (ones-matmul) use the same bf16
    exp values, so the normalization is self-consistent.
  - x streams in as 16 half-row chunks and Q/K projections run at
    query-tile granularity, so attention for the first query tile
    starts while the tail of x is still in flight.
  - output is written bf16 (host upcasts and reduces in fp32), emitted
    per query tile right after the second head-pair finishes that tile.
  - causal masks multiply on GPSIMD (otherwise idle), exp runs on ACT
    with the fully-masked prefix of both heads trimmed from the tile.
"""

import os
import sys

for _p in ("/opt/trn_rl_repo", "/root/.axon_site/_ro/trn_rl_repo"):
    if os.path.isdir(_p) and _p not in sys.path:
        sys.path.append(_p)

import ml_dtypes
import numpy as np

import concourse.bacc as bacc
import concourse.mybir as mybir
import concourse.tile as tile
from concourse.bass_utils import run_bass_kernel_spmd

F32 = mybir.dt.float32
BF16 = mybir.dt.bfloat16

B = 2          # batch
S = 2048       # sequence length
DM = 1024      # d_model
DH = 64        # d_head
NHEAD = 16     # total heads
NH = 4         # heads per core
NPAIR = 2      # head pairs per core
DC = DM // 128   # d_model chunks of 128 -> 8
KC = S // 128    # key chunks of 128 -> 16
QT = S // 512    # query tiles of 512 -> 4

# Set by test harness to capture HW profile; harmless defaults for grading.
TRACE = False
TRACE_DIR = None
LAST_EXEC_NS = None


def _build(with_bias: bool):
    nc = bacc.Bacc("TRN2", target_bir_lowering=False, debug=False)

    xT = nc.dram_tensor("xT", [DM, S], BF16, kind="ExternalInput").ap()
    wq = nc.dram_tensor("wq", [128, DC * NH * DH], BF16, kind="ExternalInput").ap()
    wk = nc.dram_tensor("wk", [128, DC * NH * DH], BF16, kind="ExternalInput").ap()
    wv = nc.dram_tensor("wv", [128, DC * NH * DH], BF16, kind="ExternalInput").ap()
    wo = nc.dram_tensor("wo", [128, NPAIR * DM], BF16, kind="ExternalInput").ap()
    mask = nc.dram_tensor("mask", [128, 128], BF16, kind="ExternalInput").ap()
    if with_bias:
        bq = nc.dram_tensor("bq", [1, NH * DH], BF16, kind="ExternalInput").ap()
        bk = nc.dram_tensor("bk", [1, NH * DH], BF16, kind="ExternalInput").ap()
        bv = nc.dram_tensor("bv", [1, NH * DH], BF16, kind="ExternalInput").ap()
    outT = nc.dram_tensor("outT", [QT, 128, DC * 512], BF16, kind="ExternalOutput").ap()

    with tile.TileContext(nc) as tc:
        with (
            tc.tile_pool(name="const", bufs=1) as cpool,
            tc.tile_pool(name="qk", bufs=1) as qkpool,
            tc.tile_pool(name="xt", bufs=16) as xtpool,
            tc.tile_pool(name="expS", bufs=2) as epool,
            tc.tile_pool(name="small", bufs=2) as spool,
            tc.tile_pool(name="zt", bufs=4) as ztpool,
            tc.tile_pool(name="out", bufs=2) as opool,
            tc.tile_pool(name="ps", bufs=1, space="PSUM") as psP,
        ):
            wo_sb = cpool.tile([128, NPAIR, DM], BF16, name="wo")
            mask_sb = cpool.tile([128, 128], BF16, name="mask")
            ones_bf = cpool.tile([128, DH], BF16, name="ones_bf")
            nc.vector.memset(ones_bf[:, :], 1.0)

            # PE warmup: dependency-chained dummy matmuls keep the PE busy
            # through the input-DMA head so the HAM clock gate releases to
            # 2.4 GHz before (and stays released until) the real work lands.
            warm = cpool.tile([128, 512], BF16, name="warm")
            nc.vector.memset(warm[:, :], 0.0)
            wps = [psP.tile([128, 512], F32, name="ps_acc", bufs=4)
                   for _ in range(4)]
            for i in range(18):
                nc.tensor.matmul(
                    wps[i % 4][:, 0:256],
                    lhsT=warm[:, 0:128],
                    rhs=warm[:, 0:256],
                    start=True,
                    stop=True,
                    skip_group_check=True,
                )
            wq_sb = cpool.tile([128, DC, NH * DH], BF16, name="wq")
            wk_sb = cpool.tile([128, DC, NH * DH], BF16, name="wk")
            wv_sb = cpool.tile([128, DC, NH * DH], BF16, name="wv")
            if with_bias:
                ones32 = cpool.tile([128, 512], BF16, name="ones32")
                nc.vector.memset(ones32[:, :], 1.0)
                bq_sb = cpool.tile([128, NH * DH], BF16, name="bq")
                bk_sb = cpool.tile([128, NH * DH], BF16, name="bk")
                bv_sb = cpool.tile([128, NH * DH], BF16, name="bv")

            qt_sb = [qkpool.tile([128, S], BF16, name=f"qt{p}") for p in range(NPAIR)]
            kt_sb = [qkpool.tile([128, S], BF16, name=f"kt{p}") for p in range(NPAIR)]
            v_sb = qkpool.tile([128, KC, NH * DH], BF16, name="v")

            # x arrives as 16 tiles: (d-model chunk c) x (seq half h).
            xh = [[xtpool.tile([128, 1024], BF16, name="xt") for _ in range(2)]
                  for _ in range(DC)]

            # ---- input DMA, all on the sync queue (keeps ACT free),
            # issue-ordered so the earliest-needed bytes stream first.
            nc.sync.dma_start(wq_sb[:, 0:4, :], wq[:, 0:1024])
            for c in range(2):
                nc.sync.dma_start(xh[c][0][:, :], xT[c * 128:(c + 1) * 128, 0:1024])
            nc.sync.dma_start(wq_sb[:, 4:8, :], wq[:, 1024:2048])
            nc.sync.dma_start(wk_sb[:, 0:4, :], wk[:, 0:1024])
            nc.sync.dma_start(wk_sb[:, 4:8, :], wk[:, 1024:2048])
            for c in range(2, DC):
                nc.sync.dma_start(xh[c][0][:, :], xT[c * 128:(c + 1) * 128, 0:1024])
            nc.sync.dma_start(wv_sb[:, :, :], wv[:, :])
            nc.sync.dma_start(mask_sb[:, :], mask[:, :])
            for c in range(DC):
                nc.sync.dma_start(xh[c][1][:, :], xT[c * 128:(c + 1) * 128, 1024:2048])
            nc.sync.dma_start(wo_sb[:, :, :], wo[:, :])
            if with_bias:
                nc.sync.dma_start(bq_sb[0:1, :], bq[:, :])
                nc.sync.dma_start(bk_sb[0:1, :], bk[:, :])
                nc.sync.dma_start(bv_sb[0:1, :], bv[:, :])

            def xq(c, j):
                """x chunk c, query-tile j columns: [128, 512]."""
                return xh[c][j // 2][:, (j % 2) * 512:(j % 2) * 512 + 512]

            def qk_proj(p, jts, head=False):
                """Project Q and K for pair p, query tiles jts, c-outer so
                every arriving x chunk feeds 2*len(jts) matmuls."""
                accs = {}
                for pj in range(2):
                    for j in jts:
                        accs[(pj, j)] = psP.tile([128, 512], F32, name="ps_acc", bufs=4)
                for c in range(DC):
                    for (pj, j), ps in accs.items():
                        w_sb = wq_sb if pj == 0 else wk_sb
                        nc.tensor.matmul(
                            ps[:, :],
                            lhsT=w_sb[:, c, p * 128:(p + 1) * 128],
                            rhs=xq(c, j),
                            start=(c == 0),
                            stop=(c == DC - 1 and not with_bias),
                        )
                if with_bias:
                    for (pj, j), ps in accs.items():
                        bias_t = bq_sb if pj == 0 else bk_sb
                        nc.tensor.matmul(
                            ps[:, :],
                            lhsT=bias_t[0:1, p * 128:(p + 1) * 128],
                            rhs=ones32[0:1, :],
                            start=False,
                            stop=True,
                        )
                for j in jts:
                    for pj in range(2):
                        ps = accs[(pj, j)]
                        dst = qt_sb[p] if pj == 0 else kt_sb[p]
                        if head and pj == 1:
                            # ACT is idle before the first exp; parallel cast
                            nc.scalar.activation(
                                dst[:, j * 512:(j + 1) * 512],
                                ps[:, :],
                                mybir.ActivationFunctionType.Copy,
                            )
                        else:
                            nc.vector.tensor_copy(
                                dst[:, j * 512:(j + 1) * 512], ps[:, :])

            def v_proj(kts):
                """Project V (natural layout) for key chunks kts; 1 bank each."""
                accs = [psP.tile([128, 512], F32, name="ps_acc", bufs=4) for _ in kts]
                for c in range(DC):
                    for i, k in enumerate(kts):
                        nc.tensor.matmul(
                            accs[i][:, :NH * DH],
                            lhsT=xh[c][k // 8][:, (k % 8) * 128:(k % 8) * 128 + 128],
                            rhs=wv_sb[:, c, :],
                            start=(c == 0),
                            stop=(c == DC - 1 and not with_bias),
                            skip_group_check=True,
                        )
                if with_bias:
                    for i in range(len(kts)):
                        nc.tensor.matmul(
                            accs[i][:, :NH * DH],
                            lhsT=ones32[0:1, 0:128],
                            rhs=bv_sb[0:1, :],
                            start=False,
                            stop=True,
                            skip_group_check=True,
                        )
                for i, k in enumerate(kts):
                    nc.vector.tensor_copy(v_sb[:, k, :], accs[i][:, :NH * DH])

            zts = {}    # (p, j) -> zt tile
            ess = {}    # (p, j) -> es tile

            def chunk_order(j):
                # diagonal chunks first so their masks run while later
                # exps stream; full chunks (no mask dep) close the chain
                return list(range(4 * j, 4 * j + 4)) + list(range(4 * j))

            def scores_phase(p, j):
                es = epool.tile([128, KC * 2 * 512], BF16, name="es")
                ess[(p, j)] = es
                for c in chunk_order(j):
                    tp = c - 4 * j
                    a = 128 * tp if tp >= 0 else 0
                    off = c * 1024
                    ps = psP.tile([128, 1024], F32, name="ps_sc", bufs=2)
                    # h0 -> ps[a:512], h1 packed at ps[512:1024-a]
                    nc.tensor.matmul(
                        ps[:, a:512],
                        lhsT=kt_sb[p][0:64, c * 128:(c + 1) * 128],
                        rhs=qt_sb[p][0:64, j * 512 + a:(j + 1) * 512],
                        start=True,
                        stop=True,
                    )
                    nc.tensor.matmul(
                        ps[:, 512:1024 - a],
                        lhsT=kt_sb[p][64:128, c * 128:(c + 1) * 128],
                        rhs=qt_sb[p][64:128, j * 512 + a:(j + 1) * 512],
                        start=True,
                        stop=True,
                    )
                    nc.scalar.activation(
                        es[:, off + a:off + 1024 - a],
                        ps[:, a:1024 - a],
                        mybir.ActivationFunctionType.Exp,
                    )
                    if tp >= 0:
                        # triangle mask on the diagonal 128-query band
                        nc.gpsimd.tensor_mul(
                            out=es[:, off + a:off + a + 128],
                            in0=es[:, off + a:off + a + 128],
                            in1=mask_sb[:, :],
                        )
                        nc.gpsimd.tensor_mul(
                            out=es[:, off + 512:off + 512 + 128],
                            in0=es[:, off + 512:off + 512 + 128],
                            in1=mask_sb[:, :],
                        )

            def pv_phase(p, j):
                # PV + column sums; h0/h1 col-packed, emitted adjacently so
                # they run concurrently in disjoint array column groups
                es = ess[(p, j)]
                order = chunk_order(j)
                ps_z2 = psP.tile([128, 512], F32, name="ps_acc", bufs=4)
                ps_s2 = psP.tile([128, 512], F32, name="ps_acc", bufs=4)
                for ci, c in enumerate(order):
                    tp = c - 4 * j
                    a = 128 * tp if tp >= 0 else 0
                    off = c * 1024
                    sl_h = (slice(off + a, off + 512),
                            slice(off + 512, off + 1024 - a))
                    for hi in range(2):
                        col = 64 * hi
                        hcore = 2 * p + hi
                        nc.tensor.matmul(
                            ps_z2[col:col + 64, a:512],
                            lhsT=v_sb[:, c, hcore * DH:(hcore + 1) * DH],
                            rhs=es[:, sl_h[hi]],
                            start=(ci == 0),
                            stop=(ci == len(order) - 1),
                            tile_position=(0, col),
                            skip_group_check=True,
                        )
                    for hi in range(2):
                        col = 64 * hi
                        nc.tensor.matmul(
                            ps_s2[col:col + 64, a:512],
                            lhsT=ones_bf[:, :],
                            rhs=es[:, sl_h[hi]],
                            start=(ci == 0),
                            stop=(ci == len(order) - 1),
                            tile_position=(0, col),
                            skip_group_check=True,
                        )
                recip = spool.tile([128, 512], F32, name="recip")
                nc.vector.reciprocal_approx_fast(recip[:, :], ps_s2[:, :])
                zt = ztpool.tile([128, 512], BF16, name=f"zt{p}")
                nc.vector.tensor_mul(zt[:, :], ps_z2[:, :], recip[:, :])
                zts[(p, j)] = zt

            def emit_wo(j):
                ot = opool.tile([128, DC * 512], BF16, name="ot")
                for d in range(DC):
                    ps = psP.tile([128, 512], F32, name="ps_acc", bufs=4)
                    for p in range(NPAIR):
                        nc.tensor.matmul(
                            ps[:, :],
                            lhsT=wo_sb[:, p, d * 128:(d + 1) * 128],
                            rhs=zts[(p, j)][:, :],
                            start=(p == 0),
                            stop=(p == NPAIR - 1),
                        )
                    if d % 2 == 0:
                        nc.vector.tensor_copy(ot[:, d * 512:(d + 1) * 512], ps[:, :])
                    else:
                        nc.scalar.activation(
                            ot[:, d * 512:(d + 1) * 512],
                            ps[:, :],
                            mybir.ActivationFunctionType.Copy,
                        )
                    if d == 3:
                        nc.sync.dma_start(outT[j][:, 0:2048], ot[:, 0:2048])
                nc.sync.dma_start(outT[j][:, 2048:4096], ot[:, 2048:4096])

            # phase pipeline: attention for (pair 0, tile 0) starts as soon
            # as the first seq half of x, wq/wk/wv and the first V quarter
            # are in; later projections fill PE while ACT runs pair-0 exps.
            qk_proj(0, [0, 1], head=True)
            scores_phase(0, 0)
            v_proj([0, 1, 2, 3])
            scores_phase(0, 1)
            pv_phase(0, 0)
            qk_proj(0, [2, 3])
            scores_phase(0, 2)
            v_proj([4, 5, 6, 7])
            pv_phase(0, 1)
            scores_phase(0, 3)
            v_proj([8, 9, 10, 11])
            pv_phase(0, 2)
            qk_proj(1, [0, 1])
            scores_phase(1, 0)
            v_proj([12, 13, 14, 15])
            pv_phase(0, 3)
            qk_proj(1, [2, 3])
            scores_phase(1, 1)
            pv_phase(1, 0)
            emit_wo(0)
            scores_phase(1, 2)
            pv_phase(1, 1)
            emit_wo(1)
            scores_phase(1, 3)
            pv_phase(1, 2)
            emit_wo(2)
            pv_phase(1, 3)
            emit_wo(3)

    nc.compile()
    return nc


_cache = {}


def _get(with_bias: bool):
    if with_bias not in _cache:
        _cache[with_bias] = _build(with_bias)
    return _cache[with_bias]


def kernel(x, W_Q, W_K, W_V, W_O, b_Q, b_K, b_V, b_O):
    global LAST_EXEC_NS
    x = np.asarray(x, dtype=np.float32)
    W_Q = np.asarray(W_Q, dtype=np.float32)
    W_K = np.asarray(W_K, dtype=np.float32)
    W_V = np.asarray(W_V, dtype=np.float32)
    W_O = np.asarray(W_O, dtype=np.float32)
    b_Q = np.asarray(b_Q, dtype=np.float32)
    b_K = np.asarray(b_K, dtype=np.float32)
    b_V = np.asarray(b_V, dtype=np.float32)
    b_O = np.asarray(b_O, dtype=np.float32)

    with_bias = bool(np.any(b_Q) or np.any(b_K) or np.any(b_V))
    nc = _get(with_bias)

    xT = np.ascontiguousarray(x.transpose(0, 2, 1))  # [B, DM, S]
    kp = np.arange(128)[:, None]
    qf = np.arange(128)[None, :]
    mask = np.where(qf >= kp, 1.0, 0.0).astype(ml_dtypes.bfloat16)

    in_maps = []
    for core in range(8):
        b, g = divmod(core, 4)
        hs = slice(NH * g, NH * g + NH)
        bf = ml_dtypes.bfloat16

        def packw(w):  # [DM, NH*DH] -> [128, DC*NH*DH] chunk-major
            return np.ascontiguousarray(
                w.reshape(DC, 128, NH * DH).transpose(1, 0, 2).reshape(128, DC * NH * DH)
            )

        m = {
            "xT": xT[b].astype(bf),
            "wq": packw((W_Q[hs] * 0.125).transpose(1, 0, 2).reshape(DM, NH * DH).astype(bf)),
            "wk": packw(W_K[hs].transpose(1, 0, 2).reshape(DM, NH * DH).astype(bf)),
            "wv": packw(W_V[hs].transpose(1, 0, 2).reshape(DM, NH * DH).astype(bf)),
            "wo": np.ascontiguousarray(
                W_O[hs].reshape(NH * DH, DM).astype(bf)
                .reshape(NPAIR, 128, DM).transpose(1, 0, 2).reshape(128, NPAIR * DM)
            ),
            "mask": mask,
        }
        if with_bias:
            m["bq"] = (b_Q[hs] * 0.125).reshape(1, NH * DH).astype(bf)
            m["bk"] = b_K[hs].reshape(1, NH * DH).astype(bf)
            m["bv"] = b_V[hs].reshape(1, NH * DH).astype(bf)
        in_maps.append(m)

    kwargs = {}
    if TRACE:
        kwargs = {"trace": True}
        if TRACE_DIR:
            kwargs["tmpdir"] = TRACE_DIR
    res = run_bass_kernel_spmd(nc, in_maps, list(range(8)), **kwargs)
    LAST_EXEC_NS = res.exec_time_ns

    out = np.empty((B, S, DM), dtype=np.float32)
    for b in range(B):
        acc = res.results[4 * b]["outT"].astype(np.float32)
        for g in range(1, 4):
            acc = acc + res.results[4 * b + g]["outT"].astype(np.float32)
        # acc: [QT, 128, DC*512]; full[d*128+p, j*512+c] = acc[j, p, d*512+c]
        full = acc.reshape(QT, 128, DC, 512).transpose(2, 1, 0, 3).reshape(DM, S)
        out[b] = full.T + b_O[None, :]
    return out


# revision 45
# speedup vs baseline: 1.0325x; 1.0325x over previous
"""Causal multi-head attention on 8 Trainium2 NeuronCores.

Sharding: core c -> batch (c // 4), head-group (c % 4) of 4 heads
(tensor-parallel over the 16 heads, data-parallel over batch=2).
Each core computes its 4 heads' contribution to the output projection;
the host sums the 4 per-head-group partials per batch (the "all-reduce")
and adds b_O.

Kernel layout notes (per core):
  - everything is computed in transposed [feature, seq] layout so the
    softmax reduction (over keys) lands on PSUM partitions and can be
    done with a ones-matmul on the PE.
  - all matmul operands are bf16 (PSUM accumulation fp32); softmax
    numerator (P@V) and denominator (ones-matmul) use the same bf16
    exp values, so the normalization is self-consistent.
  - x streams in as 16 half-row chunks and Q/K projections run at
    query-tile granularity, so attention for the first query tile
    starts while the tail of x is still in flight.
  - output is written bf16 (host upcasts and reduces in fp32), emitted
    per query tile right after the second head-pair finishes that tile.
  - causal masks multiply on GPSIMD (otherwise idle), exp runs on ACT
    with the fully-masked prefix of both heads trimmed from the tile.
"""

import os
import sys

for _p in ("/opt/trn_rl_repo", "/root/.axon_site/_ro/trn_rl_repo"):
    if os.path.isdir(_p) and _p not in sys.path:
        sys.path.append(_p)

import ml_dtypes
import numpy as np

import concourse.bacc as bacc
import concourse.mybir as mybir
import concourse.tile as tile
from concourse.bass_utils import run_bass_kernel_spmd

F32 = mybir.dt.float32
BF16 = mybir.dt.bfloat16

B = 2          # batch
S = 2048       # sequence length
DM = 1024      # d_model
DH = 64        # d_head
NHEAD = 16     # total heads
NH = 4         # heads per core
NPAIR = 2      # head pairs per core
DC = DM // 128   # d_model chunks of 128 -> 8
KC = S // 128    # key chunks of 128 -> 16
QT = S // 512    # query tiles of 512 -> 4

# Set by test harness to capture HW profile; harmless defaults for grading.
TRACE = False
TRACE_DIR = None
LAST_EXEC_NS = None


def _build(with_bias: bool):
    nc = bacc.Bacc("TRN2", target_bir_lowering=False, debug=False)

    xT = nc.dram_tensor("xT", [DM, S], BF16, kind="ExternalInput").ap()
    wq = nc.dram_tensor("wq", [128, DC * NH * DH], BF16, kind="ExternalInput").ap()
    wk = nc.dram_tensor("wk", [128, DC * NH * DH], BF16, kind="ExternalInput").ap()
    wv = nc.dram_tensor("wv", [128, DC * NH * DH], BF16, kind="ExternalInput").ap()
    wo = nc.dram_tensor("wo", [128, NPAIR * DM], BF16, kind="ExternalInput").ap()
    mask = nc.dram_tensor("mask", [128, 128], BF16, kind="ExternalInput").ap()
    if with_bias:
        bq = nc.dram_tensor("bq", [1, NH * DH], BF16, kind="ExternalInput").ap()
        bk = nc.dram_tensor("bk", [1, NH * DH], BF16, kind="ExternalInput").ap()
        bv = nc.dram_tensor("bv", [1, NH * DH], BF16, kind="ExternalInput").ap()
    outT = nc.dram_tensor("outT", [QT, 128, DC * 512], BF16, kind="ExternalOutput").ap()

    with tile.TileContext(nc) as tc:
        with (
            tc.tile_pool(name="const", bufs=1) as cpool,
            tc.tile_pool(name="qk", bufs=1) as qkpool,
            tc.tile_pool(name="xt", bufs=16) as xtpool,
            tc.tile_pool(name="expS", bufs=2) as epool,
            tc.tile_pool(name="small", bufs=2) as spool,
            tc.tile_pool(name="zt", bufs=4) as ztpool,
            tc.tile_pool(name="out", bufs=2) as opool,
            tc.tile_pool(name="ps", bufs=1, space="PSUM") as psP,
        ):
            wo_sb = cpool.tile([128, NPAIR, DM], BF16, name="wo")
            mask_sb = cpool.tile([128, 128], BF16, name="mask")
            ones_bf = cpool.tile([128, DH], BF16, name="ones_bf")
            nc.vector.memset(ones_bf[:, :], 1.0)

            # PE warmup: dependency-chained dummy matmuls keep the PE busy
            # through the input-DMA head so the HAM clock gate releases to
            # 2.4 GHz before (and stays released until) the real work lands.
            warm = cpool.tile([128, 512], BF16, name="warm")
            nc.vector.memset(warm[:, :], 0.0)
            wps = [psP.tile([128, 512], F32, name="ps_acc", bufs=4)
                   for _ in range(2)]
            for i in range(18):
                nc.tensor.matmul(
                    wps[i % 2][:, 0:256],
                    lhsT=warm[:, 0:128],
                    rhs=warm[:, 0:256],
                    start=True,
                    stop=True,
                    skip_group_check=True,
                )
            wq_sb = cpool.tile([128, DC, NH * DH], BF16, name="wq")
            wk_sb = cpool.tile([128, DC, NH * DH], BF16, name="wk")
            wv_sb = cpool.tile([128, DC, NH * DH], BF16, name="wv")
            if with_bias:
                ones32 = cpool.tile([128, 512], BF16, name="ones32")
                nc.vector.memset(ones32[:, :], 1.0)
                bq_sb = cpool.tile([128, NH * DH], BF16, name="bq")
                bk_sb = cpool.tile([128, NH * DH], BF16, name="bk")
                bv_sb = cpool.tile([128, NH * DH], BF16, name="bv")

            qt_sb = [qkpool.tile([128, S], BF16, name=f"qt{p}") for p in range(NPAIR)]
            kt_sb = [qkpool.tile([128, S], BF16, name=f"kt{p}") for p in range(NPAIR)]
            v_sb = qkpool.tile([128, KC, NH * DH], BF16, name="v")

            # x arrives as 16 tiles: (d-model chunk c) x (seq half h).
            xh = [[xtpool.tile([128, 1024], BF16, name="xt") for _ in range(2)]
                  for _ in range(DC)]

            # ---- input DMA, all on the sync queue (keeps ACT free),
            # issue-ordered so the earliest-needed bytes stream first.
            nc.sync.dma_start(wq_sb[:, 0:4, :], wq[:, 0:1024])
            for c in range(2):
                nc.sync.dma_start(xh[c][0][:, :], xT[c * 128:(c + 1) * 128, 0:1024])
            nc.sync.dma_start(wq_sb[:, 4:8, :], wq[:, 1024:2048])
            nc.sync.dma_start(wk_sb[:, 0:4, :], wk[:, 0:1024])
            nc.sync.dma_start(wk_sb[:, 4:8, :], wk[:, 1024:2048])
            for c in range(2, DC):
                nc.sync.dma_start(xh[c][0][:, :], xT[c * 128:(c + 1) * 128, 0:1024])
            nc.sync.dma_start(wv_sb[:, :, :], wv[:, :])
            nc.sync.dma_start(mask_sb[:, :], mask[:, :])
            for c in range(DC):
                nc.sync.dma_start(xh[c][1][:, :], xT[c * 128:(c + 1) * 128, 1024:2048])
            nc.sync.dma_start(wo_sb[:, :, :], wo[:, :])
            if with_bias:
                nc.sync.dma_start(bq_sb[0:1, :], bq[:, :])
                nc.sync.dma_start(bk_sb[0:1, :], bk[:, :])
                nc.sync.dma_start(bv_sb[0:1, :], bv[:, :])

            def xq(c, j):
                """x chunk c, query-tile j columns: [128, 512]."""
                return xh[c][j // 2][:, (j % 2) * 512:(j % 2) * 512 + 512]

            def qk_proj(p, jts, head=False):
                """Project Q and K for pair p, query tiles jts, c-outer so
                every arriving x chunk feeds 2*len(jts) matmuls."""
                accs = {}
                for pj in range(2):
                    for j in jts:
                        accs[(pj, j)] = psP.tile([128, 512], F32, name="ps_acc", bufs=4)
                for c in range(DC):
                    for (pj, j), ps in accs.items():
                        w_sb = wq_sb if pj == 0 else wk_sb
                        nc.tensor.matmul(
                            ps[:, :],
                            lhsT=w_sb[:, c, p * 128:(p + 1) * 128],
                            rhs=xq(c, j),
                            start=(c == 0),
                            stop=(c == DC - 1 and not with_bias),
                        )
                if with_bias:
                    for (pj, j), ps in accs.items():
                        bias_t = bq_sb if pj == 0 else bk_sb
                        nc.tensor.matmul(
                            ps[:, :],
                            lhsT=bias_t[0:1, p * 128:(p + 1) * 128],
                            rhs=ones32[0:1, :],
                            start=False,
                            stop=True,
                        )
                for j in jts:
                    for pj in range(2):
                        ps = accs[(pj, j)]
                        dst = qt_sb[p] if pj == 0 else kt_sb[p]
                        if head and pj == 1:
                            # ACT is idle before the first exp; parallel cast
                            nc.scalar.activation(
                                dst[:, j * 512:(j + 1) * 512],
                                ps[:, :],
                                mybir.ActivationFunctionType.Copy,
                            )
                        else:
                            nc.vector.tensor_copy(
                                dst[:, j * 512:(j + 1) * 512], ps[:, :])

            def v_proj(kts):
                """Project V (natural layout) for key chunks kts; 1 bank each."""
                accs = [psP.tile([128, 512], F32, name="ps_acc", bufs=4) for _ in kts]
                for c in range(DC):
                    for i, k in enumerate(kts):
                        nc.tensor.matmul(
                            accs[i][:, :NH * DH],
                            lhsT=xh[c][k // 8][:, (k % 8) * 128:(k % 8) * 128 + 128],
                            rhs=wv_sb[:, c, :],
                            start=(c == 0),
                            stop=(c == DC - 1 and not with_bias),
                            skip_group_check=True,
                        )
                if with_bias:
                    for i in range(len(kts)):
                        nc.tensor.matmul(
                            accs[i][:, :NH * DH],
                            lhsT=ones32[0:1, 0:128],
                            rhs=bv_sb[0:1, :],
                            start=False,
                            stop=True,
                            skip_group_check=True,
                        )
                for i, k in enumerate(kts):
                    nc.vector.tensor_copy(v_sb[:, k, :], accs[i][:, :NH * DH])

            zts = {}    # (p, j) -> zt tile
            ess = {}    # (p, j) -> es tile

            def chunk_order(j):
                # diagonal chunks first so their masks run while later
                # exps stream; full chunks (no mask dep) close the chain
                return list(range(4 * j, 4 * j + 4)) + list(range(4 * j))

            def scores_phase(p, j):
                es = epool.tile([128, KC * 2 * 512], BF16, name="es")
                ess[(p, j)] = es
                for c in chunk_order(j):
                    tp = c - 4 * j
                    a = 128 * tp if tp >= 0 else 0
                    off = c * 1024
                    ps = psP.tile([128, 1024], F32, name="ps_sc", bufs=2)
                    # h0 -> ps[a:512], h1 packed at ps[512:1024-a]
                    nc.tensor.matmul(
                        ps[:, a:512],
                        lhsT=kt_sb[p][0:64, c * 128:(c + 1) * 128],
                        rhs=qt_sb[p][0:64, j * 512 + a:(j + 1) * 512],
                        start=True,
                        stop=True,
                    )
                    nc.tensor.matmul(
                        ps[:, 512:1024 - a],
                        lhsT=kt_sb[p][64:128, c * 128:(c + 1) * 128],
                        rhs=qt_sb[p][64:128, j * 512 + a:(j + 1) * 512],
                        start=True,
                        stop=True,
                    )
                    nc.scalar.activation(
                        es[:, off + a:off + 1024 - a],
                        ps[:, a:1024 - a],
                        mybir.ActivationFunctionType.Exp,
                    )
                    if tp >= 0:
                        # triangle mask on the diagonal 128-query band
                        nc.gpsimd.tensor_mul(
                            out=es[:, off + a:off + a + 128],
                            in0=es[:, off + a:off + a + 128],
                            in1=mask_sb[:, :],
                        )
                        nc.gpsimd.tensor_mul(
                            out=es[:, off + 512:off + 512 + 128],
                            in0=es[:, off + 512:off + 512 + 128],
                            in1=mask_sb[:, :],
                        )

            def pv_phase(p, j):
                # PV + column sums; h0/h1 col-packed, emitted adjacently so
                # they run concurrently in disjoint array column groups
                es = ess[(p, j)]
                order = chunk_order(j)
                ps_z2 = psP.tile([128, 512], F32, name="ps_acc", bufs=4)
                ps_s2 = psP.tile([128, 512], F32, name="ps_acc", bufs=4)
                for ci, c in enumerate(order):
                    tp = c - 4 * j
                    a = 128 * tp if tp >= 0 else 0
                    off = c * 1024
                    sl_h = (slice(off + a, off + 512),
                            slice(off + 512, off + 1024 - a))
                    for hi in range(2):
                        col = 64 * hi
                        hcore = 2 * p + hi
                        nc.tensor.matmul(
                            ps_z2[col:col + 64, a:512],
                            lhsT=v_sb[:, c, hcore * DH:(hcore + 1) * DH],
                            rhs=es[:, sl_h[hi]],
                            start=(ci == 0),
                            stop=(ci == len(order) - 1),
                            tile_position=(0, col),
                            skip_group_check=True,
                        )
                    for hi in range(2):
                        col = 64 * hi
                        nc.tensor.matmul(
                            ps_s2[col:col + 64, a:512],
                            lhsT=ones_bf[:, :],
                            rhs=es[:, sl_h[hi]],
                            start=(ci == 0),
                            stop=(ci == len(order) - 1),
                            tile_position=(0, col),
                            skip_group_check=True,
                        )
                recip = spool.tile([128, 512], F32, name="recip")
                nc.vector.reciprocal_approx_fast(recip[:, :], ps_s2[:, :])
                zt = ztpool.tile([128, 512], BF16, name=f"zt{p}")
                nc.vector.tensor_mul(zt[:, :], ps_z2[:, :], recip[:, :])
                zts[(p, j)] = zt

            def emit_wo(j):
                ot = opool.tile([128, DC * 512], BF16, name="ot")
                for d in range(DC):
                    ps = psP.tile([128, 512], F32, name="ps_acc", bufs=4)
                    for p in range(NPAIR):
                        nc.tensor.matmul(
                            ps[:, :],
                            lhsT=wo_sb[:, p, d * 128:(d + 1) * 128],
                            rhs=zts[(p, j)][:, :],
                            start=(p == 0),
                            stop=(p == NPAIR - 1),
                        )
                    if d % 2 == 0:
                        nc.vector.tensor_copy(ot[:, d * 512:(d + 1) * 512], ps[:, :])
                    else:
                        nc.scalar.activation(
                            ot[:, d * 512:(d + 1) * 512],
                            ps[:, :],
                            mybir.ActivationFunctionType.Copy,
                        )
                    if d == 3:
                        nc.sync.dma_start(outT[j][:, 0:2048], ot[:, 0:2048])
                nc.sync.dma_start(outT[j][:, 2048:4096], ot[:, 2048:4096])

            # phase pipeline: attention for (pair 0, tile 0) starts as soon
            # as the first seq half of x, wq/wk/wv and the first V quarter
            # are in; later projections fill PE while ACT runs pair-0 exps.
            qk_proj(0, [0, 1], head=True)
            scores_phase(0, 0)
            v_proj([0, 1, 2, 3])
            scores_phase(0, 1)
            pv_phase(0, 0)
            qk_proj(0, [2, 3])
            scores_phase(0, 2)
            v_proj([4, 5, 6, 7])
            pv_phase(0, 1)
            scores_phase(0, 3)
            v_proj([8, 9, 10, 11])
            pv_phase(0, 2)
            qk_proj(1, [0, 1])
            scores_phase(1, 0)
            v_proj([12, 13, 14, 15])
            pv_phase(0, 3)
            qk_proj(1, [2, 3])
            scores_phase(1, 1)
            pv_phase(1, 0)
            emit_wo(0)
            scores_phase(1, 2)
            pv_phase(1, 1)
            emit_wo(1)
            scores_phase(1, 3)
            pv_phase(1, 2)
            emit_wo(2)
            pv_phase(1, 3)
            emit_wo(3)

    nc.compile()
    return nc


_cache = {}


def _get(with_bias: bool):
    if with_bias not in _cache:
        _cache[with_bias] = _build(with_bias)
    return _cache[with_bias]


def kernel(x, W_Q, W_K, W_V, W_O, b_Q, b_K, b_V, b_O):
    global LAST_EXEC_NS
    x = np.asarray(x, dtype=np.float32)
    W_Q = np.asarray(W_Q, dtype=np.float32)
    W_K = np.asarray(W_K, dtype=np.float32)
    W_V = np.asarray(W_V, dtype=np.float32)
    W_O = np.asarray(W_O, dtype=np.float32)
    b_Q = np.asarray(b_Q, dtype=np.float32)
    b_K = np.asarray(b_K, dtype=np.float32)
    b_V = np.asarray(b_V, dtype=np.float32)
    b_O = np.asarray(b_O, dtype=np.float32)

    with_bias = bool(np.any(b_Q) or np.any(b_K) or np.any(b_V))
    nc = _get(with_bias)

    xT = np.ascontiguousarray(x.transpose(0, 2, 1))  # [B, DM, S]
    kp = np.arange(128)[:, None]
    qf = np.arange(128)[None, :]
    mask = np.where(qf >= kp, 1.0, 0.0).astype(ml_dtypes.bfloat16)

    in_maps = []
    for core in range(8):
        b, g = divmod(core, 4)
        hs = slice(NH * g, NH * g + NH)
        bf = ml_dtypes.bfloat16

        def packw(w):  # [DM, NH*DH] -> [128, DC*NH*DH] chunk-major
            return np.ascontiguousarray(
                w.reshape(DC, 128, NH * DH).transpose(1, 0, 2).reshape(128, DC * NH * DH)
            )

        m = {
            "xT": xT[b].astype(bf),
            "wq": packw((W_Q[hs] * 0.125).transpose(1, 0, 2).reshape(DM, NH * DH).astype(bf)),
            "wk": packw(W_K[hs].transpose(1, 0, 2).reshape(DM, NH * DH).astype(bf)),
            "wv": packw(W_V[hs].transpose(1, 0, 2).reshape(DM, NH * DH).astype(bf)),
            "wo": np.ascontiguousarray(
                W_O[hs].reshape(NH * DH, DM).astype(bf)
                .reshape(NPAIR, 128, DM).transpose(1, 0, 2).reshape(128, NPAIR * DM)
            ),
            "mask": mask,
        }
        if with_bias:
            m["bq"] = (b_Q[hs] * 0.125).reshape(1, NH * DH).astype(bf)
            m["bk"] = b_K[hs].reshape(1, NH * DH).astype(bf)
            m["bv"] = b_V[hs].reshape(1, NH * DH).astype(bf)
        in_maps.append(m)

    kwargs = {}
    if TRACE:
        kwargs = {"trace": True}
        if TRACE_DIR:
            kwargs["tmpdir"] = TRACE_DIR
    res = run_bass_kernel_spmd(nc, in_maps, list(range(8)), **kwargs)
    LAST_EXEC_NS = res.exec_time_ns

    out = np.empty((B, S, DM), dtype=np.float32)
    for b in range(B):
        acc = res.results[4 * b]["outT"].astype(np.float32)
        for g in range(1, 4):
            acc = acc + res.results[4 * b + g]["outT"].astype(np.float32)
        # acc: [QT, 128, DC*512]; full[d*128+p, j*512+c] = acc[j, p, d*512+c]
        full = acc.reshape(QT, 128, DC, 512).transpose(2, 1, 0, 3).reshape(DM, S)
        out[b] = full.T + b_O[None, :]
    return out
